# revision 13
# baseline (speedup 1.0000x reference)
"""Trainium2 Bass kernel for the RY-encoding quantum-kernel estimator.

Math: k[b,i] = |prod_w cos((x[b,w]-xref[i,w])/2)|; out = mean_i(k) * W + b.

Uses cos(a-b) = cos a cos b + sin a sin b, so the 4-wire product expands into
a rank-16 factorization k = F @ G^T with
  F[b,s] = prod_w (bit_w(s) ? sin : cos)(x[b,w]/2)        (B,16)
  G[i,s] = prod_w (bit_w(s) ? sin : cos)(xref[i,w]/2)     (R,16)
Per core (data-parallel over batch, 8 cores x 1024 rows):
  trig on ScalarE, product tree on GPSIMD, PE transposes to get F^T/G^T,
  64 K=16 float32r matmuls into PSUM, then a fused |.|+row-sum sweep split
  between ScalarE (Abs + accum_out) and VectorE (reduce add with
  apply_absolute_value), and the tiny readout affine at the end.
"""

import numpy as np

B, R, W_DIM = 8192, 4096, 4
NCORES = 8
BS = B // NCORES          # 1024 batch rows per core
P = 128                   # partitions
BT = BS // P              # 8 batch tiles per core
RT = R // P               # 32 ref tiles
NS = 16                   # rank (2^W_DIM)
NSPAN = 2048              # psum sweep span (4 banks)
HALF_PI = float(np.pi / 2)

_NC_CACHE = None


def _split_waits(nc, limit=1):
    """Walrus in this env rejects >limit sync-waits on one instruction
    ("Too many sync wait commands").  Hoist excess waits onto freshly
    inserted same-engine NoOp carriers just before the instruction —
    engine queues are in-order, so this preserves semantics exactly."""
    import concourse.mybir as mybir

    k = 0
    for f in nc.m.functions:
        for bb in f.blocks:
            il = list(bb.instructions)
            out = []
            changed = False
            for ins in il:
                si = ins.sync_info
                ow = list(si.on_wait) if si is not None and si.on_wait else []
                if len(ow) > limit:
                    excess, keep = ow[:-limit], ow[-limit:]
                    for i in range(0, len(excess), limit):
                        nop = mybir.InstNoOp(name=f"waitnop-{k}", ins=[], outs=[])
                        k += 1
                        nop.engine = ins.engine
                        nop.sync_info = mybir.SyncInfo(
                            on_wait=excess[i : i + limit], on_update=[]
                        )
                        out.append(nop)
                    si.on_wait = keep
                    changed = True
                out.append(ins)
            if changed:
                bb.instructions = out


def _build_nc(split=True, reps=1, act_cols=1110, prep_gpsimd=True):
    import concourse.bass as bass
    import concourse.mybir as mybir
    import concourse.tile as tile
    from contextlib import ExitStack

    F32 = mybir.dt.float32
    F32R = mybir.dt.float32r
    BF16 = mybir.dt.bfloat16
    AFT = mybir.ActivationFunctionType
    ALU = mybir.AluOpType
    AX = mybir.AxisListType

    nc = bass.Bass()
    xf = nc.dram_tensor("xf", [P, BT * W_DIM], F32, kind="ExternalInput")
    rf = nc.dram_tensor("rf", [P, RT * W_DIM], F32, kind="ExternalInput")
    wb = nc.dram_tensor("wb", [P, 2], F32, kind="ExternalInput")
    out_d = nc.dram_tensor("out", [P, BT], F32, kind="ExternalOutput")
    ident_d = nc.inline_tensor(np.eye(P, dtype=np.float32), "ident128")

    with ExitStack() as ctx:
        tc = ctx.enter_context(tile.TileContext(nc))
        consts = ctx.enter_context(tc.tile_pool(name="consts", bufs=1))
        prep = ctx.enter_context(tc.tile_pool(name="prep", bufs=1))
        accp = ctx.enter_context(tc.tile_pool(name="accp", bufs=2))
        mm = ctx.enter_context(tc.tile_pool(name="mm", bufs=2, space="PSUM"))
        scr = ctx.enter_context(tc.tile_pool(name="scr", bufs=2))

        pe = nc.gpsimd if prep_gpsimd else nc.vector

        # ---- loads ----
        xf_t = consts.tile([P, BT * W_DIM], F32)
        nc.sync.dma_start(xf_t[:], xf[:])
        rf_t = consts.tile([P, RT * W_DIM], F32)
        nc.sync.dma_start(rf_t[:], rf[:])
        wb_t = consts.tile([P, 2], F32)
        nc.sync.dma_start(wb_t[:], wb[:])
        id_t = consts.tile([P, P], F32)
        nc.sync.dma_start(id_t[:], ident_d[:])

        # ---- trig (ScalarE) ----
        # cos(u/2) = sin(pi/2 - |u|/2)  (in-range for |u| <= 3pi)
        # sin(u/2) = sin(u/2)           (in-range for |u| <= 2pi)
        hpi_t = consts.tile([P, 1], F32)
        nc.gpsimd.memset(hpi_t[:], HALF_PI)

        def trig(src_t, n):
            ab = prep.tile([P, n], F32, tag=f"ab{n}")
            nc.scalar.activation(ab[:], src_t[:], AFT.Abs)
            c = prep.tile([P, n], F32, tag=f"c{n}")
            nc.scalar.activation(c[:], ab[:], AFT.Sin, scale=-0.5, bias=hpi_t[:])
            s = prep.tile([P, n], F32, tag=f"s{n}")
            nc.scalar.activation(s[:], src_t[:], AFT.Sin, scale=0.5)
            return c, s

        cosx, sinx = trig(xf_t, BT * W_DIM)
        cosr, sinr = trig(rf_t, RT * W_DIM)

        # ---- product tree: FG[p, t*16 + s], s = j23*4 + j01 ----
        def products(c, s, nt, name):
            cv = c[:].rearrange("p (t w) -> p t w", w=W_DIM)
            sv = s[:].rearrange("p (t w) -> p t w", w=W_DIM)
            p01 = prep.tile([P, nt * 4], F32, tag=f"p01{name}")
            p23 = prep.tile([P, nt * 4], F32, tag=f"p23{name}")
            p01v = p01[:].rearrange("p (t j) -> p t j", j=4)
            p23v = p23[:].rearrange("p (t j) -> p t j", j=4)
            for j in range(4):
                a0 = (sv if j & 1 else cv)[:, :, 0:1]
                a1 = (sv if j & 2 else cv)[:, :, 1:2]
                pe.tensor_mul(p01v[:, :, j : j + 1], a0, a1)
                b2 = (sv if j & 1 else cv)[:, :, 2:3]
                b3 = (sv if j & 2 else cv)[:, :, 3:4]
                pe.tensor_mul(p23v[:, :, j : j + 1], b2, b3)
            fg = prep.tile([P, nt * NS], F32, tag=f"fg{name}")
            fgv = fg[:].rearrange("p (t a b) -> p t a b", a=4, b=4)
            in0 = p01v.unsqueeze(2).broadcast_to((P, nt, 4, 4))
            in1 = p23v.unsqueeze(3).broadcast_to((P, nt, 4, 4))
            pe.tensor_mul(fgv, in0, in1)
            return fg

        F = products(cosx, sinx, BT, "f")    # (128, 128)
        G = products(cosr, sinr, RT, "g")    # (128, 512)

        # ---- transposes (PE) + regather DMAs -> fT (16,1024), gT (16,4096) ----
        fT = consts.tile([NS, BT * P], F32)
        gT = consts.tile([NS, RT * P], F32)
        tp = mm.tile([P, NSPAN], F32, tag="mm")
        # 5 chunks of (128,128): F, then G[:, c*128:(c+1)*128]
        nc.tensor.transpose(tp[:, 0:P], F[:], id_t[:])
        for c in range(4):
            nc.tensor.transpose(
                tp[:, (c + 1) * P : (c + 2) * P], G[:, c * P : (c + 1) * P], id_t[:]
            )
        # copy PSUM -> SBUF, then regather via SBUF->SBUF partition-slice DMAs:
        # stacked[t*16+s, b] -> dest[s, t*128+b]
        tpc = prep.tile([P, 5 * P], F32)
        nc.vector.tensor_copy(tpc[:], tp[:, 0 : 5 * P])
        for t in range(BT):
            nc.sync.dma_start(
                fT[:, t * P : (t + 1) * P], tpc[t * NS : (t + 1) * NS, 0:P]
            )
        for t in range(RT):
            c, tl = divmod(t, 8)
            nc.sync.dma_start(
                gT[:, t * P : (t + 1) * P],
                tpc[tl * NS : (tl + 1) * NS, (c + 1) * P : (c + 2) * P],
            )

        # ---- main loop (repeated `reps` times for differential timing) ----
        # Every psum tile is swept by BOTH engines on disjoint column ranges:
        # ScalarE Abs+accum on [0:xa], VectorE abs-reduce on [xa:NSPAN].
        # xa balances (172+xa)/1.2GHz (ACT) vs (120+NSPAN-xa)/0.96GHz (DVE).
        xa = act_cols
        for r in range(reps):
            acc_a = accp.tile([P, 2 * BT], F32, tag="acc_a")
            acc_d = accp.tile([P, 2 * BT], F32, tag="acc_d")
            for m in range(BT):
                lhsT = fT[:, m * P : (m + 1) * P].bitcast(F32R)
                for h in range(2):
                    g = m * 2 + h
                    pt = mm.tile([P, NSPAN], F32, tag="mm")
                    for j in range(4):
                        n = h * 4 + j
                        nc.tensor.matmul(
                            pt[:, j * 512 : (j + 1) * 512],
                            lhsT,
                            gT[:, n * 512 : (n + 1) * 512].bitcast(F32R),
                            start=True,
                            stop=True,
                        )
                    so = scr.tile([P, xa], BF16, tag="so")
                    nc.scalar.activation(
                        so[:], pt[:, 0:xa], AFT.Abs, accum_out=acc_a[:, g : g + 1]
                    )
                    nc.vector.tensor_reduce(
                        acc_d[:, g : g + 1],
                        pt[:, xa:NSPAN],
                        axis=AX.X,
                        op=ALU.add,
                        apply_absolute_value=True,
                    )

            # ---- readout: y = sum_g(acc_a + acc_d) * (W/R) + b ----
            stot = accp.tile([P, 2 * BT], F32, tag="stot")
            nc.vector.tensor_add(stot[:], acc_a[:], acc_d[:])
            ssum = accp.tile([P, BT], F32, tag="ssum")
            nc.vector.tensor_reduce(
                ssum[:],
                stot[:].rearrange("p (m e) -> p m e", e=2),
                axis=AX.X,
                op=ALU.add,
            )
            y = accp.tile([P, BT], F32, tag="y")
            nc.vector.tensor_scalar(
                y[:],
                ssum[:],
                wb_t[:, 0:1],
                wb_t[:, 1:2],
                op0=ALU.mult,
                op1=ALU.add,
            )
            nc.sync.dma_start(out_d[:], y[:])

    if split:
        _split_waits(nc)
    return nc


def get_nc(split=True):
    global _NC_CACHE
    if _NC_CACHE is None:
        _NC_CACHE = _build_nc(split)
    return _NC_CACHE


def make_in_maps(x, x_ref, W, b):
    x = np.ascontiguousarray(np.asarray(x, dtype=np.float32))
    x_ref = np.ascontiguousarray(np.asarray(x_ref, dtype=np.float32))
    W = np.asarray(W, dtype=np.float32)
    b = np.asarray(b, dtype=np.float32)
    # fat layout: dest[p, t*4+w] = src[t*128+p, w]
    rfm = np.ascontiguousarray(
        x_ref.reshape(RT, P, W_DIM).transpose(1, 0, 2).reshape(P, RT * W_DIM)
    )
    wbm = np.empty((P, 2), np.float32)
    wbm[:, 0] = W[0, 0] / np.float32(R)
    wbm[:, 1] = b[0]
    in_maps = []
    for c in range(NCORES):
        xs = np.ascontiguousarray(
            x[c * BS : (c + 1) * BS]
            .reshape(BT, P, W_DIM)
            .transpose(1, 0, 2)
            .reshape(P, BT * W_DIM)
        )
        in_maps.append({"xf": xs, "rf": rfm, "wb": wbm})
    return in_maps


def gather_out(results):
    # per-core out (128, 8): out[p, m] = y[batch m*128+p]
    outs = [np.asarray(r["out"], np.float32).T.reshape(BS, 1) for r in results]
    return np.concatenate(outs, axis=0)


def kernel(x, x_ref, W, b):
    from concourse.bass_utils import run_bass_kernel_spmd

    nc = get_nc()
    in_maps = make_in_maps(x, x_ref, W, b)
    res = run_bass_kernel_spmd(nc, in_maps, list(range(NCORES)))
    return gather_out(res.results)


# revision 25
# speedup vs baseline: 600.2453x; 600.2453x over previous
"""Trainium2 Bass kernel for the RY-encoding quantum-kernel estimator.

Math: k[b,i] = |prod_w cos((x[b,w]-xref[i,w])/2)|; out = mean_i(k) * W + b.

Uses cos(a-b) = cos a cos b + sin a sin b, so the 4-wire product expands into
a rank-16 factorization k = F @ G^T with
  F[b,s] = prod_w (bit_w(s) ? sin : cos)(x[b,w]/2)        (B,16)
  G[i,s] = prod_w (bit_w(s) ? sin : cos)(xref[i,w]/2)     (R,16)
Per core (data-parallel over batch, 8 cores x 1024 rows):
  trig on ScalarE, product tree on GPSIMD, PE transposes to get F^T/G^T,
  64 K=16 float32r matmuls into PSUM, then a fused |.|+row-sum sweep split
  between ScalarE (Abs + accum_out) and VectorE (reduce add with
  apply_absolute_value), and the tiny readout affine at the end.
"""

import numpy as np

B, R, W_DIM = 8192, 4096, 4
NCORES = 8
BS = B // NCORES          # 1024 batch rows per core
P = 128                   # partitions
BT = BS // P              # 8 batch tiles per core
RT = R // P               # 32 ref tiles
NS = 16                   # rank (2^W_DIM)
NSPAN = 2048              # psum sweep span (4 banks)
HALF_PI = float(np.pi / 2)

_NC_CACHE = None


def _split_waits(nc, limit=1):
    """Walrus in this env rejects >limit sync-waits on one instruction
    ("Too many sync wait commands").  Hoist excess waits onto freshly
    inserted same-engine NoOp carriers just before the instruction —
    engine queues are in-order, so this preserves semantics exactly."""
    import concourse.mybir as mybir

    k = 0
    for f in nc.m.functions:
        for bb in f.blocks:
            il = list(bb.instructions)
            out = []
            changed = False
            for ins in il:
                si = ins.sync_info
                ow = list(si.on_wait) if si is not None and si.on_wait else []
                if len(ow) > limit:
                    excess, keep = ow[:-limit], ow[-limit:]
                    for i in range(0, len(excess), limit):
                        nop = mybir.InstNoOp(name=f"waitnop-{k}", ins=[], outs=[])
                        k += 1
                        nop.engine = ins.engine
                        nop.sync_info = mybir.SyncInfo(
                            on_wait=excess[i : i + limit], on_update=[]
                        )
                        out.append(nop)
                    si.on_wait = keep
                    changed = True
                out.append(ins)
            if changed:
                bb.instructions = out


def _build_nc(
    split=True,
    reps=1,
    act_cols=1128,
    prep_gpsimd=True,
    pack=True,
    sweep_mode="alt",
    act_tiles=8,
):
    import concourse.bass as bass
    import concourse.mybir as mybir
    import concourse.tile as tile
    from contextlib import ExitStack

    F32 = mybir.dt.float32
    F32R = mybir.dt.float32r
    BF16 = mybir.dt.bfloat16
    AFT = mybir.ActivationFunctionType
    ALU = mybir.AluOpType
    AX = mybir.AxisListType

    nc = bass.Bass()
    xf = nc.dram_tensor("xf", [P, BT * W_DIM], F32, kind="ExternalInput")
    rf = nc.dram_tensor("rf", [P, RT * W_DIM], F32, kind="ExternalInput")
    wb = nc.dram_tensor("wb", [P, 2], F32, kind="ExternalInput")
    out_d = nc.dram_tensor("out", [P, BT], F32, kind="ExternalOutput")
    ident_d = nc.inline_tensor(np.eye(P, dtype=np.float32), "ident128")

    with ExitStack() as ctx:
        tc = ctx.enter_context(tile.TileContext(nc))
        consts = ctx.enter_context(tc.tile_pool(name="consts", bufs=1))
        prep = ctx.enter_context(tc.tile_pool(name="prep", bufs=1))
        accp = ctx.enter_context(tc.tile_pool(name="accp", bufs=2))
        mm = ctx.enter_context(tc.tile_pool(name="mm", bufs=2, space="PSUM"))
        scr = ctx.enter_context(tc.tile_pool(name="scr", bufs=2))

        pe = nc.gpsimd if prep_gpsimd else nc.vector

        # ---- loads ----
        xf_t = consts.tile([P, BT * W_DIM], F32)
        nc.sync.dma_start(xf_t[:], xf[:])
        rf_t = consts.tile([P, RT * W_DIM], F32)
        nc.sync.dma_start(rf_t[:], rf[:])
        wb_t = consts.tile([P, 2], F32)
        nc.sync.dma_start(wb_t[:], wb[:])
        id_t = consts.tile([P, P], F32)
        nc.sync.dma_start(id_t[:], ident_d[:])

        # ---- trig (ScalarE) ----
        # cos(u/2) = sin(pi/2 - |u|/2)  (in-range for |u| <= 3pi)
        # sin(u/2) = sin(u/2)           (in-range for |u| <= 2pi)
        hpi_t = consts.tile([P, 1], F32)
        nc.gpsimd.memset(hpi_t[:], HALF_PI)

        def trig(src_t, n):
            ab = prep.tile([P, n], F32, tag=f"ab{n}")
            nc.scalar.activation(ab[:], src_t[:], AFT.Abs)
            c = prep.tile([P, n], F32, tag=f"c{n}")
            nc.scalar.activation(c[:], ab[:], AFT.Sin, scale=-0.5, bias=hpi_t[:])
            s = prep.tile([P, n], F32, tag=f"s{n}")
            nc.scalar.activation(s[:], src_t[:], AFT.Sin, scale=0.5)
            return c, s

        cosx, sinx = trig(xf_t, BT * W_DIM)
        cosr, sinr = trig(rf_t, RT * W_DIM)

        # ---- product tree: FG[p, t*16 + s], s = j23*4 + j01 ----
        def products(c, s, nt, name):
            cv = c[:].rearrange("p (t w) -> p t w", w=W_DIM)
            sv = s[:].rearrange("p (t w) -> p t w", w=W_DIM)
            p01 = prep.tile([P, nt * 4], F32, tag=f"p01{name}")
            p23 = prep.tile([P, nt * 4], F32, tag=f"p23{name}")
            p01v = p01[:].rearrange("p (t j) -> p t j", j=4)
            p23v = p23[:].rearrange("p (t j) -> p t j", j=4)
            for j in range(4):
                a0 = (sv if j & 1 else cv)[:, :, 0:1]
                a1 = (sv if j & 2 else cv)[:, :, 1:2]
                pe.tensor_mul(p01v[:, :, j : j + 1], a0, a1)
                b2 = (sv if j & 1 else cv)[:, :, 2:3]
                b3 = (sv if j & 2 else cv)[:, :, 3:4]
                pe.tensor_mul(p23v[:, :, j : j + 1], b2, b3)
            fg = prep.tile([P, nt * NS], F32, tag=f"fg{name}")
            fgv = fg[:].rearrange("p (t a b) -> p t a b", a=4, b=4)
            in0 = p01v.unsqueeze(2).broadcast_to((P, nt, 4, 4))
            in1 = p23v.unsqueeze(3).broadcast_to((P, nt, 4, 4))
            pe.tensor_mul(fgv, in0, in1)
            return fg

        F = products(cosx, sinx, BT, "f")    # (128, 128)
        G = products(cosr, sinr, RT, "g")    # (128, 512)

        # ---- transposes (PE) + regather DMAs -> fT (16,1024), gT (16,4096) ----
        # With pack=True the tiles are (128, .) and F^T/G^T are replicated at
        # partition bases 0/32/64/96 so 4 K=16 matmuls run concurrently in
        # distinct PE row-groups (tile_position auto-derives from base).
        nrep = 4 if pack else 1
        fT = consts.tile([P if pack else NS, BT * P], F32)
        gT = consts.tile([P if pack else NS, RT * P], F32)
        tp = mm.tile([P, NSPAN], F32, tag="mm")
        # 5 chunks of (128,128): F, then G[:, c*128:(c+1)*128]
        nc.tensor.transpose(tp[:, 0:P], F[:], id_t[:])
        for c in range(4):
            nc.tensor.transpose(
                tp[:, (c + 1) * P : (c + 2) * P], G[:, c * P : (c + 1) * P], id_t[:]
            )
        # copy PSUM -> SBUF, then regather via SBUF->SBUF partition-slice DMAs:
        # stacked[t*16+s, b] -> dest[s, t*128+b]
        tpc = prep.tile([P, 5 * P], F32)
        nc.vector.tensor_copy(tpc[:], tp[:, 0 : 5 * P])
        for t in range(BT):
            nc.sync.dma_start(
                fT[0:NS, t * P : (t + 1) * P], tpc[t * NS : (t + 1) * NS, 0:P]
            )
        for t in range(RT):
            c, tl = divmod(t, 8)
            nc.sync.dma_start(
                gT[0:NS, t * P : (t + 1) * P],
                tpc[tl * NS : (tl + 1) * NS, (c + 1) * P : (c + 2) * P],
            )
        if pack:
            for j in range(1, 4):
                nc.sync.dma_start(fT[j * 32 : j * 32 + NS, :], fT[0:NS, :])
                nc.sync.dma_start(gT[j * 32 : j * 32 + NS, :], gT[0:NS, :])

        # ---- main loop (repeated `reps` times for differential timing) ----
        # Every psum tile is swept by BOTH engines on disjoint column ranges:
        # ScalarE Abs+accum on [0:xa], VectorE abs-reduce on [xa:NSPAN].
        # xa balances (172+xa)/1.2GHz (ACT) vs (120+NSPAN-xa)/0.96GHz (DVE).
        xa = act_cols
        for r in range(reps):
            acc_a = accp.tile([P, 2 * BT], F32, tag="acc_a")
            acc_d = accp.tile([P, 2 * BT], F32, tag="acc_d")
            for m in range(BT):
                for h in range(2):
                    g = m * 2 + h
                    pt = mm.tile([P, NSPAN], F32, tag="mm")
                    for j in range(4):
                        n = h * 4 + j
                        base = (j % nrep) * 32
                        lhsT = fT[base : base + NS, m * P : (m + 1) * P].bitcast(F32R)
                        nc.tensor.matmul(
                            pt[:, j * 512 : (j + 1) * 512],
                            lhsT,
                            gT[base : base + NS, n * 512 : (n + 1) * 512].bitcast(F32R),
                            start=True,
                            stop=True,
                            tile_position=(base, 0),
                        )
                    if sweep_mode == "split":
                        so = scr.tile([P, xa], BF16, tag="so")
                        nc.scalar.activation(
                            so[:], pt[:, 0:xa], AFT.Abs, accum_out=acc_a[:, g : g + 1]
                        )
                        nc.vector.tensor_reduce(
                            acc_d[:, g : g + 1],
                            pt[:, xa:NSPAN],
                            axis=AX.X,
                            op=ALU.add,
                            apply_absolute_value=True,
                        )
                    elif (g * act_tiles) // 16 != ((g + 1) * act_tiles) // 16:
                        so = scr.tile([P, NSPAN], BF16, tag="so")
                        nc.scalar.activation(
                            so[:], pt[:], AFT.Abs, accum_out=acc_a[:, g : g + 1]
                        )
                        nc.gpsimd.memset(acc_d[:, g : g + 1], 0.0)
                    else:
                        nc.vector.tensor_reduce(
                            acc_d[:, g : g + 1],
                            pt[:],
                            axis=AX.X,
                            op=ALU.add,
                            apply_absolute_value=True,
                        )
                        nc.gpsimd.memset(acc_a[:, g : g + 1], 0.0)

            # ---- readout (GPSIMD; keeps ACT/DVE free): ----
            # y = sum_g(acc_a + acc_d) * (W/R) + b
            stot = accp.tile([P, 2 * BT], F32, tag="stot")
            nc.gpsimd.tensor_add(stot[:], acc_a[:], acc_d[:])
            sv = stot[:].rearrange("p (m e) -> p m e", e=2)
            ssum = accp.tile([P, BT], F32, tag="ssum")
            sso = ssum[:].unsqueeze(2)
            nc.gpsimd.tensor_add(sso, sv[:, :, 0:1], sv[:, :, 1:2])
            y = accp.tile([P, BT], F32, tag="y")
            nc.gpsimd.tensor_scalar(
                y[:],
                ssum[:],
                wb_t[:, 0:1],
                wb_t[:, 1:2],
                op0=ALU.mult,
                op1=ALU.add,
            )
            nc.sync.dma_start(out_d[:], y[:])

    if split:
        _split_waits(nc)
    return nc


def get_nc(split=True):
    global _NC_CACHE
    if _NC_CACHE is None:
        _NC_CACHE = _build_nc(split)
    return _NC_CACHE


def make_in_maps(x, x_ref, W, b):
    x = np.ascontiguousarray(np.asarray(x, dtype=np.float32))
    x_ref = np.ascontiguousarray(np.asarray(x_ref, dtype=np.float32))
    W = np.asarray(W, dtype=np.float32)
    b = np.asarray(b, dtype=np.float32)
    # fat layout: dest[p, t*4+w] = src[t*128+p, w]
    rfm = np.ascontiguousarray(
        x_ref.reshape(RT, P, W_DIM).transpose(1, 0, 2).reshape(P, RT * W_DIM)
    )
    wbm = np.empty((P, 2), np.float32)
    wbm[:, 0] = W[0, 0] / np.float32(R)
    wbm[:, 1] = b[0]
    in_maps = []
    for c in range(NCORES):
        xs = np.ascontiguousarray(
            x[c * BS : (c + 1) * BS]
            .reshape(BT, P, W_DIM)
            .transpose(1, 0, 2)
            .reshape(P, BT * W_DIM)
        )
        in_maps.append({"xf": xs, "rf": rfm, "wb": wbm})
    return in_maps


def gather_out(results):
    # per-core out (128, 8): out[p, m] = y[batch m*128+p]
    outs = [np.asarray(r["out"], np.float32).T.reshape(BS, 1) for r in results]
    return np.concatenate(outs, axis=0)


def kernel(x, x_ref, W, b):
    from concourse.bass_utils import run_bass_kernel_spmd

    nc = get_nc()
    in_maps = make_in_maps(x, x_ref, W, b)
    res = run_bass_kernel_spmd(nc, in_maps, list(range(NCORES)))
    return gather_out(res.results)
